# revision 3
# baseline (speedup 1.0000x reference)
"""LoRA layer kernel for Trainium2, SPMD across 8 NeuronCores.

Computes: out[b,s,h,d] = x[b,s,:] @ W_orig[:,h,d] + SCALE * (x @ A) @ B[:,h,d]

Strategy (data-parallel over tokens, per the sharding hint's DP branch):
  - Fold LoRA into the weights on the host: W_eff = W + (SCALE*A) @ B
    (exact by associativity; a 33 MFLOP host-side GEMM vs the 68.7 GFLOP
    main matmul which stays on device).
  - Cast x and W_eff to fp16 on the host: halves DMA traffic and runs the
    PE at 1 col/cycle (4x the fp32 rate) with fp32 PSUM accumulation.
    Output is written fp16 and upcast on the host (error ~5e-4 << 2e-2).
  - Shard x over tokens (8192 -> 1024 per core); W_eff replicated.
  - Per core: out[1024, 2048] = x_slice @ W_eff, K=2048 contraction in
    16 k-tiles of 128. Token tiles run in PAIRS sharing all 8 PSUM banks
    with the k-loop OUTER within a pair, so the first pair's compute
    (27 us) overlaps the W_eff stream-in (24 us) k-tile by k-tile.

Head/tail trimming (v3):
  - 12 warm-up matmuls on zeroed scratch run during the DMA fill window:
    they flip the PE HAM clock-gate to 8/8 before real data arrives, so
    no real matmul pays the 1.2 GHz cold rate.
  - DMA issue is split across both HWDGE rings: W streams on the sync
    ring, x (first tile quartered for an earlier first matmul) + outputs
    on the scalar ring, halving time-to-first-matmul.
  - The final token tile's output is evacuated and DMA'd in 4 chunks
    alternating across both rings to shorten the post-matmul tail.
"""

import numpy as np

# Problem shapes (hardcoded per contract - kernel.py must be self-contained)
B, S, H = 4, 2048, 2048
NH, HD = 16, 128
N = NH * HD            # 2048 output features
RANK = 4
ALPHA = 4.0
SCALE = ALPHA / RANK   # 1.0
NCORES = 8
TOK = B * S            # 8192 tokens total
TPC = TOK // NCORES    # 1024 tokens per core

P = 128                # SBUF partitions
KT = H // P            # 16 contraction tiles
TT = TPC // P          # 8 token tiles per core
CH = 512               # psum chunk width (one fp32 PSUM bank)
NCH = N // CH          # 4 chunks

_CACHE = {}


def _build_program():
    import concourse.mybir as mybir
    import concourse.tile as tile
    from concourse import bacc

    f16 = mybir.dt.float16
    f32 = mybir.dt.float32

    nc = bacc.Bacc(None, target_bir_lowering=False, debug=False)

    xt = nc.dram_tensor("xt", [TT, P, KT, P], f16, kind="ExternalInput")
    w = nc.dram_tensor("w", [P, KT, N], f16, kind="ExternalInput")
    out = nc.dram_tensor("out", [TT, P, N], f16, kind="ExternalOutput")

    with tile.TileContext(nc) as tc:
        with (
            tc.tile_pool(name="cpool", bufs=1) as cpool,
            tc.tile_pool(name="wpool", bufs=1) as wpool,
            tc.tile_pool(name="xpool", bufs=4) as xpool,
            tc.tile_pool(name="opool", bufs=4) as opool,
            tc.tile_pool(name="psum", bufs=8, space="PSUM") as psum,
        ):
            # --- PE warm-up: zeroed scratch matmuls flip HAM to 8/8 while
            # the first DMAs are still in flight.
            wa = cpool.tile([P, P], f16, tag="wa", name="warm_a")
            wb = cpool.tile([P, CH], f16, tag="wb", name="warm_b")
            nc.vector.memset(wa[:], 0.0)
            nc.vector.memset(wb[:], 0.0)
            warm = [
                psum.tile([P, CH], f32, tag="ps", name=f"warm_{i}")
                for i in range(8)
            ]
            for i in range(12):
                nc.tensor.matmul(warm[i % 8][:], wa[:], wb[:],
                                 start=True, stop=True)

            # --- x token tiles: scalar-engine ring. First tile arrives in
            # quarters so matmul 0 starts as early as possible.
            x_tiles = {}

            def load_x(t, split=1):
                xr = xpool.tile([P, KT, P], f16, tag="x", name=f"x_{t}")
                kq = KT // split
                for q in range(split):
                    nc.scalar.dma_start(
                        xr[:, q * kq:(q + 1) * kq, :],
                        xt[t, :, q * kq:(q + 1) * kq, :])
                x_tiles[t] = xr

            load_x(0, split=4)
            load_x(1)

            # --- W_eff k-tiles: resident, streamed on the sync ring so the
            # first pair's k-loop can chase the arrival front.
            w_tiles = []
            for k in range(KT):
                wk = wpool.tile([P, N], f16, tag=f"w{k}", name=f"w_{k}")
                nc.sync.dma_start(wk[:], w[:, k, :])
                w_tiles.append(wk)

            for pr in range(TT // 2):
                ta, tb = 2 * pr, 2 * pr + 1
                if ta + 2 < TT:
                    load_x(ta + 2)
                if tb + 2 < TT:
                    load_x(tb + 2)
                pss = {
                    (t, c): psum.tile([P, CH], f32, tag="ps",
                                      name=f"ps_{t}_{c}")
                    for t in (ta, tb) for c in range(NCH)
                }
                for k in range(KT):
                    for t in (ta, tb):
                        lhsT = x_tiles[t][:, k, :]
                        for c in range(NCH):
                            nc.tensor.matmul(
                                pss[(t, c)][:],
                                lhsT,
                                w_tiles[k][:, c * CH:(c + 1) * CH],
                                start=(k == 0), stop=(k == KT - 1),
                            )
                last = pr == TT // 2 - 1
                for t in (ta, tb):
                    ot = opool.tile([P, N], f16, tag="o", name=f"o_{t}")
                    for c in range(NCH):
                        # split evacuation across both PSUM-capable engines
                        sl = ot[:, c * CH:(c + 1) * CH]
                        if c % 2 == 0:
                            nc.vector.tensor_copy(sl, pss[(t, c)][:])
                        else:
                            nc.scalar.copy(sl, pss[(t, c)][:])
                        if last and t == tb:
                            # final tile: chunked output DMA on both rings
                            dst = out[t, :, c * CH:(c + 1) * CH]
                            eng = nc.sync if c % 2 == 0 else nc.scalar
                            eng.dma_start(dst, sl)
                    if not (last and t == tb):
                        nc.scalar.dma_start(out[t], ot[:])
                for t in (ta, tb):
                    x_tiles.pop(t)

    nc.compile()
    return nc


def _prep_inputs(x, W_orig, A_kernel, B_kernel):
    x = np.asarray(x, dtype=np.float32)
    W_orig = np.asarray(W_orig, dtype=np.float32)
    A_kernel = np.asarray(A_kernel, dtype=np.float32)
    B_kernel = np.asarray(B_kernel, dtype=np.float32)

    # Fold the rank-4 LoRA path into the dense weight (exact algebra):
    # out = x @ (W + SCALE * A @ B)
    w_eff = W_orig.reshape(H, N) + (SCALE * A_kernel) @ B_kernel.reshape(RANK, N)
    # device layout [p, k, n] with h = k*128 + p
    w_dev = np.ascontiguousarray(
        w_eff.reshape(KT, P, N).transpose(1, 0, 2)).astype(np.float16)

    x2 = x.reshape(TOK, H)
    in_maps = []
    for i in range(NCORES):
        xs = x2[i * TPC:(i + 1) * TPC]                      # [1024, 2048]
        # [t, tt, k, p] -> [t, p, k, tt] so the contraction dim is the
        # SBUF partition dim and each DMA partition-line is contiguous
        xd = np.ascontiguousarray(
            xs.reshape(TT, P, KT, P).transpose(0, 3, 2, 1)).astype(np.float16)
        in_maps.append({"xt": xd, "w": w_dev})
    return in_maps


def kernel(x, W_orig, A_kernel, B_kernel):
    from concourse.bass_utils import run_bass_kernel_spmd

    if "nc" not in _CACHE:
        _CACHE["nc"] = _build_program()
    nc = _CACHE["nc"]

    in_maps = _prep_inputs(x, W_orig, A_kernel, B_kernel)
    res = run_bass_kernel_spmd(nc, in_maps, list(range(NCORES)))
    parts = [
        res.results[i]["out"].reshape(TPC, N).astype(np.float32)
        for i in range(NCORES)
    ]
    full = np.concatenate(parts, axis=0)                    # [TOK, N]
    return full.reshape(B, S, NH, HD)


# revision 5
# speedup vs baseline: 1.0256x; 1.0256x over previous
"""LoRA layer kernel for Trainium2, SPMD across 8 NeuronCores.

Computes: out[b,s,h,d] = x[b,s,:] @ W_orig[:,h,d] + SCALE * (x @ A) @ B[:,h,d]

Strategy (data-parallel over tokens, per the sharding hint's DP branch):
  - Fold LoRA into the weights on the host: W_eff = W + (SCALE*A) @ B
    (exact by associativity; a 33 MFLOP host-side GEMM vs the 68.7 GFLOP
    main matmul which stays on device).
  - Cast x and W_eff to fp16 on the host: halves DMA traffic and runs the
    PE at 1 col/cycle (4x the fp32 rate) with fp32 PSUM accumulation.
    Output is written fp16 and upcast on the host (error ~5e-4 << 2e-2).
  - Shard x over tokens (8192 -> 1024 per core); W_eff replicated.
  - Per core: out[1024, 2048] = x_slice @ W_eff, K=2048 contraction in
    16 k-tiles of 128. Token tiles run in PAIRS sharing all 8 PSUM banks
    with the k-loop OUTER within a pair, so pair-0's compute rate
    (1.73 us per k-tile) tracks the W_eff stream-in rate (1.43 us per
    k-tile) and the weight load hides behind the matmul stream.
  - All input DMA goes on ONE HWDGE ring (sync engine): the ring's FIFO
    is the priority order. Rings share HBM bandwidth, so a second ring
    does not add throughput - it just lets late traffic starve urgent
    traffic (measured). Order = first-need order: x0/w0 halves
    interleaved for the earliest first matmul, x1, then the W stream,
    then the remaining x tiles. Output DMAs ride the scalar ring;
    the final tile's output is chunked across both rings to cut the
    post-matmul tail.
"""

import numpy as np

# Problem shapes (hardcoded per contract - kernel.py must be self-contained)
B, S, H = 4, 2048, 2048
NH, HD = 16, 128
N = NH * HD            # 2048 output features
RANK = 4
ALPHA = 4.0
SCALE = ALPHA / RANK   # 1.0
NCORES = 8
TOK = B * S            # 8192 tokens total
TPC = TOK // NCORES    # 1024 tokens per core

P = 128                # SBUF partitions
KT = H // P            # 16 contraction tiles
TT = TPC // P          # 8 token tiles per core
CH = 512               # psum chunk width (one fp32 PSUM bank)
NCH = N // CH          # 4 chunks

_CACHE = {}


def _build_program():
    import concourse.mybir as mybir
    import concourse.tile as tile
    from concourse import bacc

    f16 = mybir.dt.float16
    f32 = mybir.dt.float32

    nc = bacc.Bacc(None, target_bir_lowering=False, debug=False)

    xt = nc.dram_tensor("xt", [TT, P, KT, P], f16, kind="ExternalInput")
    w = nc.dram_tensor("w", [P, KT, N], f16, kind="ExternalInput")
    out = nc.dram_tensor("out", [TT, P, N], f16, kind="ExternalOutput")

    with tile.TileContext(nc) as tc:
        with (
            tc.tile_pool(name="wpool", bufs=1) as wpool,
            tc.tile_pool(name="xpool", bufs=TT) as xpool,
            tc.tile_pool(name="opool", bufs=4) as opool,
            tc.tile_pool(name="psum", bufs=8, space="PSUM") as psum,
        ):
            x_tiles = {}
            w_tiles = {}
            KH = KT // 2

            def x_tile(t):
                xr = xpool.tile([P, KT, P], f16, tag="x", name=f"x_{t}")
                x_tiles[t] = xr
                return xr

            def w_tile(k):
                wk = wpool.tile([P, N], f16, tag=f"w{k}", name=f"w_{k}")
                w_tiles[k] = wk
                return wk

            # Input stream on the sync ring, FIFO in first-need order.
            x0 = x_tile(0)
            w0 = w_tile(0)
            nc.sync.dma_start(x0[:, :KH, :], xt[0, :, :KH, :])
            nc.sync.dma_start(w0[:, :N // 2], w[:, 0, :N // 2])
            nc.sync.dma_start(w0[:, N // 2:], w[:, 0, N // 2:])
            x1 = x_tile(1)
            nc.sync.dma_start(x1[:, :KH, :], xt[1, :, :KH, :])
            for k in (1, 2):
                nc.sync.dma_start(w_tile(k)[:], w[:, k, :])
            nc.sync.dma_start(x1[:, KH:, :], xt[1, :, KH:, :])
            nc.sync.dma_start(x0[:, KH:, :], xt[0, :, KH:, :])
            for k in range(3, KT):
                nc.sync.dma_start(w_tile(k)[:], w[:, k, :])
            for t in range(2, TT):
                nc.sync.dma_start(x_tile(t)[:], xt[t])

            for pr in range(TT // 2):
                ta, tb = 2 * pr, 2 * pr + 1
                pss = {
                    (t, c): psum.tile([P, CH], f32, tag="ps",
                                      name=f"ps_{t}_{c}")
                    for t in (ta, tb) for c in range(NCH)
                }
                for k in range(KT):
                    for t in (ta, tb):
                        lhsT = x_tiles[t][:, k, :]
                        for c in range(NCH):
                            nc.tensor.matmul(
                                pss[(t, c)][:],
                                lhsT,
                                w_tiles[k][:, c * CH:(c + 1) * CH],
                                start=(k == 0), stop=(k == KT - 1),
                            )
                last = pr == TT // 2 - 1
                for t in (ta, tb):
                    ot = opool.tile([P, N], f16, tag="o", name=f"o_{t}")
                    for c in range(NCH):
                        # split evacuation across both PSUM-capable engines
                        sl = ot[:, c * CH:(c + 1) * CH]
                        if c % 2 == 0:
                            nc.vector.tensor_copy(sl, pss[(t, c)][:])
                        else:
                            nc.scalar.copy(sl, pss[(t, c)][:])
                        if last and t == tb:
                            # final tile: chunked output DMA on both rings
                            dst = out[t, :, c * CH:(c + 1) * CH]
                            eng = nc.sync if c % 2 == 0 else nc.scalar
                            eng.dma_start(dst, sl)
                    if not (last and t == tb):
                        nc.scalar.dma_start(out[t], ot[:])

    nc.compile()
    return nc


def _prep_inputs(x, W_orig, A_kernel, B_kernel):
    x = np.asarray(x, dtype=np.float32)
    W_orig = np.asarray(W_orig, dtype=np.float32)
    A_kernel = np.asarray(A_kernel, dtype=np.float32)
    B_kernel = np.asarray(B_kernel, dtype=np.float32)

    # Fold the rank-4 LoRA path into the dense weight (exact algebra):
    # out = x @ (W + SCALE * A @ B)
    w_eff = W_orig.reshape(H, N) + (SCALE * A_kernel) @ B_kernel.reshape(RANK, N)
    # device layout [p, k, n] with h = k*128 + p
    w_dev = np.ascontiguousarray(
        w_eff.reshape(KT, P, N).transpose(1, 0, 2)).astype(np.float16)

    x2 = x.reshape(TOK, H)
    in_maps = []
    for i in range(NCORES):
        xs = x2[i * TPC:(i + 1) * TPC]                      # [1024, 2048]
        # [t, tt, k, p] -> [t, p, k, tt] so the contraction dim is the
        # SBUF partition dim and each DMA partition-line is contiguous
        xd = np.ascontiguousarray(
            xs.reshape(TT, P, KT, P).transpose(0, 3, 2, 1)).astype(np.float16)
        in_maps.append({"xt": xd, "w": w_dev})
    return in_maps


def kernel(x, W_orig, A_kernel, B_kernel):
    from concourse.bass_utils import run_bass_kernel_spmd

    if "nc" not in _CACHE:
        _CACHE["nc"] = _build_program()
    nc = _CACHE["nc"]

    in_maps = _prep_inputs(x, W_orig, A_kernel, B_kernel)
    res = run_bass_kernel_spmd(nc, in_maps, list(range(NCORES)))
    parts = [
        res.results[i]["out"].reshape(TPC, N).astype(np.float32)
        for i in range(NCORES)
    ]
    full = np.concatenate(parts, axis=0)                    # [TOK, N]
    return full.reshape(B, S, NH, HD)


# revision 8
# speedup vs baseline: 1.0309x; 1.0052x over previous
"""LoRA layer kernel for Trainium2, SPMD across 8 NeuronCores.

Computes: out[b,s,h,d] = x[b,s,:] @ W_orig[:,h,d] + SCALE * (x @ A) @ B[:,h,d]

Strategy (data-parallel over tokens, per the sharding hint's DP branch):
  - Fold LoRA into the weights on the host: W_eff = W + (SCALE*A) @ B
    (exact by associativity; a 33 MFLOP host-side GEMM vs the 68.7 GFLOP
    main matmul which stays on device).
  - Cast x and W_eff to fp16 on the host: halves DMA traffic and runs the
    PE at 1 col/cycle (4x the fp32 rate) with fp32 PSUM accumulation.
    Output is written fp16 and upcast on the host (error ~5e-4 << 2e-2).
  - Shard x over tokens (8192 -> 1024 per core); W_eff replicated.
  - Per core: out[1024, 2048] = x_slice @ W_eff, K=2048 contraction in
    16 k-tiles of 128. Token tiles run in PAIRS sharing all 8 PSUM banks
    with the k-loop OUTER within a pair, so pair-0's compute rate
    (1.73 us per k-tile) tracks the W_eff stream-in rate (1.43 us per
    k-tile) and the weight load hides behind the matmul stream.
  - All input DMA goes on ONE HWDGE ring (sync engine): the ring's FIFO
    is the priority order. Rings share HBM bandwidth, so a second ring
    does not add throughput - it just lets late traffic starve urgent
    traffic (measured). Order = first-need order: x0/w0 halves
    interleaved for the earliest first matmul, x1, then the W stream,
    then the remaining x tiles. Output DMAs ride the scalar ring;
    the final tile's output is chunked across both rings to cut the
    post-matmul tail.
"""

import numpy as np

# Problem shapes (hardcoded per contract - kernel.py must be self-contained)
B, S, H = 4, 2048, 2048
NH, HD = 16, 128
N = NH * HD            # 2048 output features
RANK = 4
ALPHA = 4.0
SCALE = ALPHA / RANK   # 1.0
NCORES = 8
TOK = B * S            # 8192 tokens total
TPC = TOK // NCORES    # 1024 tokens per core

P = 128                # SBUF partitions
KT = H // P            # 16 contraction tiles
TT = TPC // P          # 8 token tiles per core
CH = 512               # psum chunk width (one fp32 PSUM bank)
NCH = N // CH          # 4 chunks

_CACHE = {}


def _build_program():
    import concourse.mybir as mybir
    import concourse.tile as tile
    from concourse import bacc

    f16 = mybir.dt.float16
    f32 = mybir.dt.float32

    nc = bacc.Bacc(None, target_bir_lowering=False, debug=False)

    xt = nc.dram_tensor("xt", [TT, P, KT, P], f16, kind="ExternalInput")
    w = nc.dram_tensor("w", [P, KT, N], f16, kind="ExternalInput")
    out = nc.dram_tensor("out", [TT, P, N], f16, kind="ExternalOutput")

    with tile.TileContext(nc) as tc:
        with (
            tc.tile_pool(name="cpool", bufs=1) as cpool,
            tc.tile_pool(name="wpool", bufs=1) as wpool,
            tc.tile_pool(name="xpool", bufs=TT) as xpool,
            tc.tile_pool(name="opool", bufs=4) as opool,
            tc.tile_pool(name="psum", bufs=8, space="PSUM") as psum,
        ):
            x_tiles = {}
            w_tiles = {}
            KH = KT // 2

            def x_tile(t):
                xr = xpool.tile([P, KT, P], f16, tag="x", name=f"x_{t}")
                x_tiles[t] = xr
                return xr

            def w_tile(k):
                wk = wpool.tile([P, N], f16, tag=f"w{k}", name=f"w_{k}")
                w_tiles[k] = wk
                return wk

            # Zeroed scratch feeding "filler" matmuls: they add exact zeros
            # into live PSUM banks, keeping the PE busy (and the HAM clock
            # un-throttled) while the first input DMAs are still landing.
            zs = cpool.tile([P, P], f16, tag="zs", name="zscr")
            ws = cpool.tile([P, CH], f16, tag="ws", name="wscr")
            nc.vector.memset(zs[:], 0.0)
            nc.vector.memset(ws[:], 0.0)

            # Input stream on the sync ring, FIFO in first-need order.
            x0 = x_tile(0)
            w0 = w_tile(0)
            x1 = x_tile(1)
            nc.sync.dma_start(x0[:, :4, :], xt[0, :, :4, :])
            nc.sync.dma_start(w0[:, :N // 2], w[:, 0, :N // 2])
            nc.sync.dma_start(w0[:, N // 2:], w[:, 0, N // 2:])
            nc.sync.dma_start(x1[:, :KH, :], xt[1, :, :KH, :])
            for k in (1, 2):
                nc.sync.dma_start(w_tile(k)[:], w[:, k, :])
            nc.sync.dma_start(x0[:, 4:, :], xt[0, :, 4:, :])
            nc.sync.dma_start(x1[:, KH:, :], xt[1, :, KH:, :])
            for k in range(3, KT):
                nc.sync.dma_start(w_tile(k)[:], w[:, k, :])
            for t in range(2, TT):
                nc.sync.dma_start(x_tile(t)[:], xt[t])

            for pr in range(TT // 2):
                ta, tb = 2 * pr, 2 * pr + 1
                pss = {
                    (t, c): psum.tile([P, CH], f32, tag="ps",
                                      name=f"ps_{t}_{c}")
                    for t in (ta, tb) for c in range(NCH)
                }
                if pr == 0:
                    # pre-fillers: PE busy from the end of the preamble;
                    # flips HAM to full clock before real data lands
                    for i in range(5):
                        nc.tensor.matmul(pss[(ta, i % NCH)][:], zs[:], ws[:],
                                         start=True, stop=True)
                for k in range(KT):
                    for t in (ta, tb):
                        lhsT = x_tiles[t][:, k, :]
                        for c in range(NCH):
                            nc.tensor.matmul(
                                pss[(t, c)][:],
                                lhsT,
                                w_tiles[k][:, c * CH:(c + 1) * CH],
                                start=(k == 0), stop=(k == KT - 1),
                            )
                    if pr == 0 and k == 0:
                        # bridge the x1/w1 arrival gaps with zero-adding
                        # matmuls into the open accumulation groups
                        for i in range(2):
                            nc.tensor.matmul(pss[(ta, i)][:], zs[:], ws[:],
                                             start=False, stop=False)
                    if pr == 0 and k == 1:
                        for i in range(3):
                            nc.tensor.matmul(pss[(tb, i)][:], zs[:], ws[:],
                                             start=False, stop=False)
                last = pr == TT // 2 - 1
                for t in (ta, tb):
                    ot = opool.tile([P, N], f16, tag="o", name=f"o_{t}")
                    for c in range(NCH):
                        # split evacuation across both PSUM-capable engines
                        sl = ot[:, c * CH:(c + 1) * CH]
                        if c % 2 == 0:
                            nc.vector.tensor_copy(sl, pss[(t, c)][:])
                        else:
                            nc.scalar.copy(sl, pss[(t, c)][:])
                        if last:
                            # final pair: chunked output DMA on both rings,
                            # each chunk issued right after its copy
                            dst = out[t, :, c * CH:(c + 1) * CH]
                            eng = nc.sync if c % 2 == 0 else nc.scalar
                            eng.dma_start(dst, sl)
                    if not last:
                        nc.scalar.dma_start(out[t], ot[:])

    nc.compile()
    return nc


def _prep_inputs(x, W_orig, A_kernel, B_kernel):
    x = np.asarray(x, dtype=np.float32)
    W_orig = np.asarray(W_orig, dtype=np.float32)
    A_kernel = np.asarray(A_kernel, dtype=np.float32)
    B_kernel = np.asarray(B_kernel, dtype=np.float32)

    # Fold the rank-4 LoRA path into the dense weight (exact algebra):
    # out = x @ (W + SCALE * A @ B)
    w_eff = W_orig.reshape(H, N) + (SCALE * A_kernel) @ B_kernel.reshape(RANK, N)
    # device layout [p, k, n] with h = k*128 + p
    w_dev = np.ascontiguousarray(
        w_eff.reshape(KT, P, N).transpose(1, 0, 2)).astype(np.float16)

    x2 = x.reshape(TOK, H)
    in_maps = []
    for i in range(NCORES):
        xs = x2[i * TPC:(i + 1) * TPC]                      # [1024, 2048]
        # [t, tt, k, p] -> [t, p, k, tt] so the contraction dim is the
        # SBUF partition dim and each DMA partition-line is contiguous
        xd = np.ascontiguousarray(
            xs.reshape(TT, P, KT, P).transpose(0, 3, 2, 1)).astype(np.float16)
        in_maps.append({"xt": xd, "w": w_dev})
    return in_maps


def kernel(x, W_orig, A_kernel, B_kernel):
    from concourse.bass_utils import run_bass_kernel_spmd

    if "nc" not in _CACHE:
        _CACHE["nc"] = _build_program()
    nc = _CACHE["nc"]

    in_maps = _prep_inputs(x, W_orig, A_kernel, B_kernel)
    res = run_bass_kernel_spmd(nc, in_maps, list(range(NCORES)))
    parts = [
        res.results[i]["out"].reshape(TPC, N).astype(np.float32)
        for i in range(NCORES)
    ]
    full = np.concatenate(parts, axis=0)                    # [TOK, N]
    return full.reshape(B, S, NH, HD)
